# revision 24
# baseline (speedup 1.0000x reference)
"""Gated max/avg 2x2 pooling kernel for Trainium2 (8 NeuronCores, SPMD) — v3.

Reference computation (per 2x2 window over [B, H, W, C], stride 2):
    x1 = max(window), x2 = mean(window)
    xs = sum_ij mask[i, j] * window[i, j]   (per channel)
    z  = sigmoid(xs)
    out = z * x1 + (1 - z) * x2

Engine split (all scalar operands ride as instruction immediates):
  ACT : f32->f16 deinterleaving cast (2 instr/tile), q3 = rB*Fa,
        t4 = 0.25*s, sigmoid
  DVE : max tree (2 TT), sum tree (2 TT), xs pair-Horner
        (2 TS + fused (Ap||Bp) TT + xsp TT), d/g/o blend (3 TT)
  DMA : f32 in (separate per-parity staging tiles so each frees for the
        i+2 prefetch at its own cast), f16 out (host upcasts)

xs pair-Horner: A' = rE*Ea + Eb, B' = rO*Oa + Ob computed as ONE
TT@2*fdo against a strided (Eb||Ob) view; xsp = rB*Fa + Fb;
z = sigmoid(f * xsp). The (a, b) slice order is chosen on the host so
every ratio has |r| <= 1; f is the remaining carrier scale, applied
for free by the ACT sigmoid's scale immediate.

Perf notes (hardware-measured): the kernel is DVE-throughput-bound
(~129us busy), not DMA-bound (~105us). GPSIMD elementwise ops slow
concurrent DVE ops ~2.7x via the shared SBUF port, so GPSIMD stays
idle on purpose. DVE TT on 64-element-run strided views runs at full
packed rate. tensor_reduce/pool windowed reductions and 1x custom DVE
ops are all slower than the stock TT/TS mix; moving the q1/q2 scales
to ACT delays the cast chain. enable_asserts=False trims ~5us of
per-engine preamble. The DVE clock varies ~20% between runs (p-state);
compare timings via the median TT@2048 duration (~1218ns fast).
"""

import numpy as np

import concourse.bacc as bacc
import concourse.mybir as mybir
import concourse.tile as tile
from concourse.bass_utils import run_bass_kernel_spmd

F32 = mybir.dt.float32
F16 = mybir.dt.float16

B, H, W, C = 16, 256, 256, 64
N_CORES = 8
BPC = B // N_CORES          # batches per core
HO = H // 2                 # 128 output rows = SBUF partitions
NQ = 4                      # w-quarters per row
WQ = W // NQ                # input w per macro-tile (64)

LAST_EXEC_NS = None
LAST_RESULTS = None

_PROGRAM_CACHE = {}


def _build_program(bpc, ho, nq, wq, ch, plan):
    """Build + compile the single-core Bass/Tile program (SPMD-shared).

    plan = (swapE, swapO, swapF, rE, rO, rB, f): slice assignment for the
    pair-Horner plus the ratio/carrier values, all baked as instruction
    immediates (skips the scalar-value loads a PTR operand needs).
    """
    from contextlib import ExitStack

    assert ho == 128, "partition dim must be 128"
    swapE, swapO, swapF, rE, rO, rB, fca = plan
    fd_in = wq * ch            # free dim of an E/O tile (4096)
    wo = wq // 2               # output w per macro-tile
    fd_out = wo * ch           # free dim of output tile (2048)

    nc = bacc.Bacc(
        "TRN2",
        target_bir_lowering=False,
        debug=False,
        enable_asserts=False,
        num_devices=N_CORES,
    )

    x = nc.dram_tensor("x", [bpc, ho, 2, nq, fd_in], F32, kind="ExternalInput")
    out = nc.dram_tensor("out", [bpc, ho, nq, fd_out], F16, kind="ExternalOutput")
    x_ap = x.ap()
    out_ap = out.ap()

    alu = mybir.AluOpType

    with tile.TileContext(nc) as tc, ExitStack() as ctx:
        pool_io = ctx.enter_context(tc.tile_pool(name="io", bufs=2))
        pool_y = ctx.enter_context(tc.tile_pool(name="y16", bufs=2))
        pool_big = ctx.enter_context(tc.tile_pool(name="big", bufs=1))
        pool_tmp = ctx.enter_context(tc.tile_pool(name="tmp", bufs=1))
        pool_gps = ctx.enter_context(tc.tile_pool(name="gps", bufs=2))
        pool_out = ctx.enter_context(tc.tile_pool(name="outp", bufs=2))

        def emit_load(b, q, w_lo, w_hi, dve_cast=False):
            """Stage 1: input DMA + ACT cast for one tile.

            dma_ways: extra w-wise split of each row-parity's DMA; >1 for
            the ramp tiles, where single-queue DMA latency gates startup."""
            nw = w_hi - w_lo
            fde = nw * 2 * ch
            # separate per-parity staging tiles: each frees for the i+2
            # prefetch as soon as ITS cast is done, not both
            E32 = pool_io.tile([128, fde], F32, tag="E32")
            O32 = pool_io.tile([128, fde], F32, tag="O32")
            src = x_ap[b, :, :, q, :].rearrange(
                "p r (w c) -> p r w c", c=2 * ch
            )[:, :, w_lo : w_lo + nw, :]
            nc.sync.dma_start(
                E32[:].rearrange("p (w c) -> p w c", c=2 * ch), src[:, 0])
            nc.sync.dma_start(
                O32[:].rearrange("p (w c) -> p w c", c=2 * ch), src[:, 1])
            Y = pool_y.tile([128, 2 * fde], F16, tag="Y")
            # cast + column-deinterleave: write order (r, e, w, c) while
            # reading DMA order (r, w, e, c); downstream slices contiguous.
            # One cast per row-parity keeps the APs within 3 free dims.
            Yw = Y[:].rearrange(
                "p (r e w c) -> p r w e c", r=2, e=2, c=ch
            )
            Ein = E32[:].rearrange("p (w e c) -> p w e c", e=2, c=ch)
            Oin = O32[:].rearrange("p (w e c) -> p w e c", e=2, c=ch)
            if dve_cast:
                # ramp tiles: DVE is idle waiting on this cast anyway, and
                # casts ~1.7x faster than ACT; skips two cross-engine hops
                nc.vector.tensor_copy(Yw[:, 0], Ein)
                nc.vector.tensor_copy(Yw[:, 1], Oin)
            else:
                nc.scalar.copy(Yw[:, 0], Ein)
                nc.scalar.copy(Yw[:, 1], Oin)
            return dict(b=b, q=q, w_lo=w_lo, nw=nw, fde=fde, fdo=nw * ch, Y=Y)

        def emit_compute(h, ramp=False):
            """Stage 2: DVE trees + sigmoid + GPS blend + output DMA."""
            b, q, w_lo, nw = h["b"], h["q"], h["w_lo"], h["nw"]
            fde, fdo, Y = h["fde"], h["fdo"], h["Y"]

            def tmp(tag, fd=fdo, dt=F16, pool=pool_tmp):
                t = pool.tile([128, fd], dt, tag=tag)
                return t

            Ef = Y[:, 0:fde]
            Of = Y[:, fde : 2 * fde]
            # deinterleaved layout: all four window slices are contiguous
            Ee, Eo = Y[:, 0:fdo], Y[:, fdo : 2 * fdo]
            Oe, Oo = Y[:, fde : fde + fdo], Y[:, fde + fdo : 2 * fde]

            def v(t):
                return t[:].rearrange("p (w c) -> p w c", c=ch)

            # xs chain first so the sigmoid's input is ready early; the
            # independent max/sum trees fill the ACT-latency windows
            Ea, Eb = (Eo, Ee) if swapE else (Ee, Eo)
            Oa, Ob = (Oo, Oe) if swapO else (Oe, Oo)
            # q12 = (rE*Ea || rO*Oa); one TT then adds (Eb || Ob), so the
            # two pair-combines share a single @2*fdo instruction
            q12 = tmp("q12", fd=fde, pool=pool_gps)
            nc.vector.tensor_scalar_mul(q12[:, 0:fdo], Ea, float(rE))
            nc.vector.tensor_scalar_mul(q12[:, fdo:fde], Oa, float(rO))
            # (Eb || Ob) as one strided view over Y's four fdo-slices
            sEb = (0 if swapE else 1)
            sOb = 2 + (0 if swapO else 1)
            Ys = Y[:].rearrange("p (s x) -> p s x", s=4)
            ebob = Ys[:, slice(sEb, sOb + 1, sOb - sEb)]
            AB = tmp("AB", fd=fde, pool=pool_gps)
            nc.vector.tensor_add(
                AB[:].rearrange("p (s x) -> p s x", s=2), q12[:].rearrange(
                    "p (s x) -> p s x", s=2), ebob,
            )
            Ap = AB[:, 0:fdo]
            Bp = AB[:, fdo:fde]
            Fa, Fb = (Ap, Bp) if swapF else (Bp, Ap)
            q3 = tmp("q3", pool=pool_gps)
            if ramp:
                nc.vector.tensor_scalar_mul(q3[:], Fa, float(rB))
            else:
                nc.scalar.mul(q3[:], Fa, float(rB))

            # big vertical combines run while ACT computes q3
            M1 = tmp("M1", fd=fde, pool=pool_big)
            nc.vector.tensor_max(M1[:], Ef, Of)
            S1 = tmp("S1", fd=fde, pool=pool_big)
            nc.vector.tensor_add(S1[:], Ef, Of)

            xsp = tmp("xsp", pool=pool_gps)
            nc.vector.tensor_add(xsp[:], q3[:], Fb)
            z = tmp("z", pool=pool_gps)
            nc.scalar.activation(
                z[:], xsp[:],
                mybir.ActivationFunctionType.Sigmoid,
                scale=float(fca),
            )

            # horizontal reduces + blend while ACT computes the sigmoid
            x1 = tmp("x1")
            nc.vector.tensor_max(x1[:], M1[:, 0:fdo], M1[:, fdo:fde])
            s = tmp("s", pool=pool_big)
            nc.vector.tensor_add(s[:], S1[:, 0:fdo], S1[:, fdo:fde])
            t4 = tmp("t4", pool=pool_gps)
            if ramp:
                nc.vector.tensor_scalar_mul(t4[:], s[:], 0.25)
            else:
                nc.scalar.mul(t4[:], s[:], 0.25)
            d = tmp("d", pool=pool_gps)
            nc.vector.tensor_sub(d[:], x1[:], t4[:])
            g = tmp("g", pool=pool_big)
            nc.vector.tensor_mul(g[:], z[:], d[:])
            o = tmp("o", pool=pool_out)
            nc.vector.tensor_add(o[:], t4[:], g[:])

            dst = out_ap[b, :, q, :].rearrange("p (w c) -> p w c", c=ch)
            nc.sync.dma_start(
                dst[:, w_lo : w_lo + nw, :],
                o[:].rearrange("p (w c) -> p w c", c=ch),
            )

        wo_q = wq // 2  # output w-pairs per quarter
        n_macro = bpc * nq
        tiles = []
        for b in range(bpc):
            for qq in range(nq):
                first = not tiles
                last = b == bpc - 1 and qq == nq - 1
                if first:
                    tiles.append((b, qq, 0, wo_q // 4))
                    tiles.append((b, qq, wo_q // 4, wo_q // 2))
                    tiles.append((b, qq, wo_q // 2, wo_q))
                elif last:
                    tiles.append((b, qq, 0, wo_q // 2))
                    tiles.append((b, qq, wo_q // 2, 3 * wo_q // 4))
                    tiles.append((b, qq, 3 * wo_q // 4, wo_q))
                else:
                    tiles.append((b, qq, 0, wo_q))
        pending = emit_load(*tiles[0], dve_cast=True)
        for i in range(len(tiles)):
            nxt = (
                emit_load(*tiles[i + 1], dve_cast=(i <= 1))
                if i + 1 < len(tiles)
                else None
            )
            emit_compute(pending, ramp=(i < 3))
            pending = nxt

    nc.compile()
    return nc


def _get_program(bpc, ho, nq, wq, ch, plan):
    key = (bpc, ho, nq, wq, ch, plan)
    if key not in _PROGRAM_CACHE:
        _PROGRAM_CACHE[key] = _build_program(bpc, ho, nq, wq, ch, plan)
    return _PROGRAM_CACHE[key]


def _mask_plan(mask):
    """Derive the plan so every ratio has |r| <= 1.

    xs = m00*Ee + m01*Eo + m10*Oe + m11*Oo
       = cE*(rE*Ea + Eb) + cO*(rO*Oa + Ob)
    with (Ea, Eb) = (Ee, Eo) or swapped so |rE| <= 1 (cE = the larger-|.|
    coefficient of the E pair), likewise the O pair. Final:
    xsp = rB*Fa + Fb with Fb the pair whose carrier |c| is larger;
    f = carrier of Fb, rB = other carrier / f.
    """
    m = np.asarray(mask, np.float64).reshape(-1)  # m00, m01, m10, m11
    mE = (m[0], m[1])
    mO = (m[2], m[3])

    def pair(coeffs):
        ca, cb = coeffs  # coeff of the 'even' slice, coeff of the 'odd' slice
        # swapped=False: A' = r*even + odd, carrier = cb (odd coeff), r = ca/cb
        # swapped=True:  A' = r*odd + even, carrier = ca, r = cb/ca
        if abs(ca) <= abs(cb):
            carrier = cb
            r = ca / cb if cb != 0.0 else 0.0
            return False, r, carrier
        carrier = ca
        r = cb / ca if ca != 0.0 else 0.0
        return True, r, carrier

    swapE, rE, cE = pair(mE)
    swapO, rO, cO = pair(mO)
    # xs = cE*A' + cO*B'; scale the smaller carrier
    if abs(cO) <= abs(cE):
        swapF = False  # scale B' (Fa = Bp), carrier f = cE
        f = cE
        rB = cO / cE if cE != 0.0 else 0.0
    else:
        swapF = True   # scale A'
        f = cO
        rB = cE / cO if cO != 0.0 else 0.0

    # ratios ride as float32 instruction immediates; round-trip through
    # float32 so the compile-key is exactly what the program computes
    rE32, rO32, rB32, f32v = (np.float32(v) for v in (rE, rO, rB, f))
    return (
        swapE, swapO, swapF,
        float(rE32), float(rO32), float(rB32), float(f32v),
    )


def kernel(x, mask):
    import os

    global LAST_EXEC_NS, LAST_RESULTS

    x = np.asarray(x)
    mask = np.asarray(mask)
    assert x.shape == (B, H, W, C), x.shape
    in_dtype = x.dtype

    plan = _mask_plan(mask)
    nc = _get_program(BPC, HO, NQ, WQ, C, plan)

    xv = np.ascontiguousarray(x, np.float32).reshape(B, HO, 2, NQ, WQ * C)

    in_maps = [
        {"x": xv[i * BPC : (i + 1) * BPC]} for i in range(N_CORES)
    ]

    trace = os.environ.get("KERNEL_TRACE", "0") == "1"
    res = run_bass_kernel_spmd(
        nc, in_maps, core_ids=list(range(N_CORES)), trace=trace
    )
    LAST_EXEC_NS = res.exec_time_ns
    LAST_RESULTS = res

    parts = [
        r["out"].reshape(BPC, HO, NQ, WQ // 2, C).reshape(BPC, HO, W // 2, C)
        for r in res.results
    ]
    full = np.concatenate(parts, axis=0)
    return full.astype(in_dtype, copy=False)


def _numpy_reference(x, mask):
    xr = x.reshape(x.shape[0], x.shape[1] // 2, 2, x.shape[2] // 2, 2, x.shape[3])
    x1 = xr.max(axis=(2, 4))
    x2 = xr.mean(axis=(2, 4))
    xs = np.einsum("bhiwjc,ij->bhwc", xr, mask)
    z = 1.0 / (1.0 + np.exp(-xs))
    return z * x1 + (1.0 - z) * x2


if __name__ == "__main__":
    # Small-scale CoreSim self-test (no hardware needed).
    from concourse.bass_interp import CoreSim

    rng = np.random.default_rng(0)
    for trial in range(4):
        bpc_s, nq_s, wq_s = 1, 1, 8
        h_s, w_s = 256, nq_s * wq_s
        xs_np = rng.standard_normal((bpc_s, h_s, w_s, C)).astype(np.float32)
        mask_np = (rng.standard_normal((2, 2)) * 0.5).astype(np.float32)

        plan_s = _mask_plan(mask_np)
        nc = _build_program(bpc_s, 128, nq_s, wq_s, C, plan_s)
        sim = CoreSim(nc, trace=False)
        sim.tensor("x")[:] = xs_np.reshape(bpc_s, 128, 2, nq_s, wq_s * C)
        sim.simulate()
        got = (
            sim.tensor("out")
            .astype(np.float64)
            .reshape(bpc_s, 128, nq_s, wq_s // 2, C)
            .reshape(bpc_s, 128, w_s // 2, C)
        )
        want = _numpy_reference(xs_np.astype(np.float64), mask_np.astype(np.float64))
        err = np.abs(got - want)
        rel = err.max() / np.abs(want).max()
        print(f"trial {trial} plan={plan_s} mask={mask_np.reshape(-1)} "
              f"max abs {err.max():.2e} rel {rel:.2e}")
        assert rel < 5e-3, rel
    print("PASS")



# revision 27
# speedup vs baseline: 1.1710x; 1.1710x over previous
"""Gated max/avg 2x2 pooling kernel for Trainium2 (8 NeuronCores, SPMD) — v2.

Reference computation (per 2x2 window over [B, H, W, C], stride 2):
    x1 = max(window), x2 = mean(window)
    xs = sum_ij mask[i, j] * window[i, j]   (per channel)
    z  = sigmoid(xs)
    out = z * x1 + (1 - z) * x2

Engine split (measured rates: DVE TT 1.0u, TS 0.61u, STT/custom 2.0u;
ACT 1.7u + 440ns/instr; GPSIMD 3.3u):
  ACT : f32->f16 cast (1 instr/tile) + sigmoid
  DVE : max tree (2 TT), sum tree (2 TT), xs pair-Horner (3 TS + 3 TT),
        t = 0.25*s (TS), d = x1 - t (TT)
  GPS : g = z*d, o = t + g   (two light ops on the otherwise idle engine)
  DMA : f32 in, f16 out (host upcasts)

xs pair-Horner: A' = rE*Ea + Eb, B' = rO*Oa + Ob, xsp = rB*Bp + Ap,
z = sigmoid(f * xsp), with (a, b) per pair and the (Bp, Ap) order chosen
on the host so every ratio has |r| <= 1; f is the remaining carrier
scale, applied for free by the ACT sigmoid.
"""

import numpy as np

import concourse.bacc as bacc
import concourse.mybir as mybir
import concourse.tile as tile
from concourse.bass_utils import run_bass_kernel_spmd

F32 = mybir.dt.float32
F16 = mybir.dt.float16

B, H, W, C = 16, 256, 256, 64
N_CORES = 8
BPC = B // N_CORES          # batches per core
HO = H // 2                 # 128 output rows = SBUF partitions
NQ = 4                      # w-quarters per row
WQ = W // NQ                # input w per macro-tile (64)

LAST_EXEC_NS = None
LAST_RESULTS = None

_PROGRAM_CACHE = {}


def _build_program(bpc, ho, nq, wq, ch, plan):
    """Build + compile the single-core Bass/Tile program (SPMD-shared).

    plan = (swapE, swapO, swapF, rE, rO, rB, f): slice assignment for the
    pair-Horner plus the ratio/carrier values, all baked as instruction
    immediates (skips the scalar-value loads a PTR operand needs).
    """
    from contextlib import ExitStack

    assert ho == 128, "partition dim must be 128"
    swapE, swapO, swapF, rE, rO, rB, fca = plan
    fd_in = wq * ch            # free dim of an E/O tile (4096)
    wo = wq // 2               # output w per macro-tile
    fd_out = wo * ch           # free dim of output tile (2048)

    nc = bacc.Bacc(
        "TRN2",
        target_bir_lowering=False,
        debug=False,
        enable_asserts=False,
        num_devices=N_CORES,
    )

    x = nc.dram_tensor("x", [bpc, ho, 2, nq, fd_in], F32, kind="ExternalInput")
    out = nc.dram_tensor("out", [bpc, ho, nq, fd_out], F16, kind="ExternalOutput")
    x_ap = x.ap()
    out_ap = out.ap()

    alu = mybir.AluOpType

    with tile.TileContext(nc) as tc, ExitStack() as ctx:
        pool_io = ctx.enter_context(tc.tile_pool(name="io", bufs=2))
        pool_y = ctx.enter_context(tc.tile_pool(name="y16", bufs=2))
        pool_big = ctx.enter_context(tc.tile_pool(name="big", bufs=1))
        pool_tmp = ctx.enter_context(tc.tile_pool(name="tmp", bufs=1))
        pool_gps = ctx.enter_context(tc.tile_pool(name="gps", bufs=2))
        pool_out = ctx.enter_context(tc.tile_pool(name="outp", bufs=2))

        def emit_load(b, q, w_lo, w_hi, dve_cast=False):
            """Stage 1: input DMA + ACT cast for one tile.

            dma_ways: extra w-wise split of each row-parity's DMA; >1 for
            the ramp tiles, where single-queue DMA latency gates startup."""
            nw = w_hi - w_lo
            fde = nw * 2 * ch
            # separate per-parity staging tiles: each frees for the i+2
            # prefetch as soon as ITS cast is done, not both
            E32 = pool_io.tile([128, fde], F32, tag="E32")
            O32 = pool_io.tile([128, fde], F32, tag="O32")
            src = x_ap[b, :, :, q, :].rearrange(
                "p r (w c) -> p r w c", c=2 * ch
            )[:, :, w_lo : w_lo + nw, :]
            nc.sync.dma_start(
                E32[:].rearrange("p (w c) -> p w c", c=2 * ch), src[:, 0])
            nc.sync.dma_start(
                O32[:].rearrange("p (w c) -> p w c", c=2 * ch), src[:, 1])
            Y = pool_y.tile([128, 2 * fde], F16, tag="Y")
            # cast + column-deinterleave: write order (r, e, w, c) while
            # reading DMA order (r, w, e, c); downstream slices contiguous.
            # One cast per row-parity keeps the APs within 3 free dims.
            Yw = Y[:].rearrange(
                "p (r e w c) -> p r w e c", r=2, e=2, c=ch
            )
            Ein = E32[:].rearrange("p (w e c) -> p w e c", e=2, c=ch)
            Oin = O32[:].rearrange("p (w e c) -> p w e c", e=2, c=ch)
            if dve_cast:
                # ramp tiles: DVE is idle waiting on this cast anyway, and
                # casts ~1.7x faster than ACT; skips two cross-engine hops
                nc.vector.tensor_copy(Yw[:, 0], Ein)
                nc.vector.tensor_copy(Yw[:, 1], Oin)
            else:
                nc.scalar.copy(Yw[:, 0], Ein)
                nc.scalar.copy(Yw[:, 1], Oin)
            return dict(b=b, q=q, w_lo=w_lo, nw=nw, fde=fde, fdo=nw * ch, Y=Y)

        def emit_compute(h, ramp=False):
            """Stage 2: DVE trees + sigmoid + GPS blend + output DMA."""
            b, q, w_lo, nw = h["b"], h["q"], h["w_lo"], h["nw"]
            fde, fdo, Y = h["fde"], h["fdo"], h["Y"]

            def tmp(tag, fd=fdo, dt=F16, pool=pool_tmp):
                t = pool.tile([128, fd], dt, tag=tag)
                return t

            Ef = Y[:, 0:fde]
            Of = Y[:, fde : 2 * fde]
            # deinterleaved layout: all four window slices are contiguous
            Ee, Eo = Y[:, 0:fdo], Y[:, fdo : 2 * fdo]
            Oe, Oo = Y[:, fde : fde + fdo], Y[:, fde + fdo : 2 * fde]

            def v(t):
                return t[:].rearrange("p (w c) -> p w c", c=ch)

            # xs chain first so the sigmoid's input is ready early; the
            # independent max/sum trees fill the ACT-latency windows
            Ea, Eb = (Eo, Ee) if swapE else (Ee, Eo)
            Oa, Ob = (Oo, Oe) if swapO else (Oe, Oo)
            # q12 = (rE*Ea || rO*Oa); one TT then adds (Eb || Ob), so the
            # two pair-combines share a single @2*fdo instruction
            q12 = tmp("q12", fd=fde, pool=pool_gps)
            nc.vector.tensor_scalar_mul(q12[:, 0:fdo], Ea, float(rE))
            nc.vector.tensor_scalar_mul(q12[:, fdo:fde], Oa, float(rO))
            # (Eb || Ob) as one strided view over Y's four fdo-slices
            sEb = (0 if swapE else 1)
            sOb = 2 + (0 if swapO else 1)
            Ys = Y[:].rearrange("p (s x) -> p s x", s=4)
            ebob = Ys[:, slice(sEb, sOb + 1, sOb - sEb)]
            AB = tmp("AB", fd=fde, pool=pool_gps)
            nc.vector.tensor_add(
                AB[:].rearrange("p (s x) -> p s x", s=2), q12[:].rearrange(
                    "p (s x) -> p s x", s=2), ebob,
            )
            Ap = AB[:, 0:fdo]
            Bp = AB[:, fdo:fde]
            Fa, Fb = (Ap, Bp) if swapF else (Bp, Ap)
            q3 = tmp("q3", pool=pool_gps)
            if ramp:
                nc.vector.tensor_scalar_mul(q3[:], Fa, float(rB))
            else:
                nc.scalar.mul(q3[:], Fa, float(rB))

            # big vertical combines run while ACT computes q3
            M1 = tmp("M1", fd=fde, pool=pool_big)
            nc.vector.tensor_max(M1[:], Ef, Of)
            S1 = tmp("S1", fd=fde, pool=pool_big)
            nc.vector.tensor_add(S1[:], Ef, Of)

            xsp = tmp("xsp", pool=pool_gps)
            nc.vector.tensor_add(xsp[:], q3[:], Fb)
            z = tmp("z", pool=pool_gps)
            nc.scalar.activation(
                z[:], xsp[:],
                mybir.ActivationFunctionType.Sigmoid,
                scale=float(fca),
            )

            # horizontal reduces + blend while ACT computes the sigmoid
            x1 = tmp("x1")
            nc.vector.tensor_max(x1[:], M1[:, 0:fdo], M1[:, fdo:fde])
            s = tmp("s", pool=pool_big)
            nc.vector.tensor_add(s[:], S1[:, 0:fdo], S1[:, fdo:fde])
            t4 = tmp("t4", pool=pool_gps)
            if ramp:
                nc.vector.tensor_scalar_mul(t4[:], s[:], 0.25)
            else:
                nc.scalar.mul(t4[:], s[:], 0.25)
            d = tmp("d", pool=pool_gps)
            nc.vector.tensor_sub(d[:], x1[:], t4[:])
            g = tmp("g", pool=pool_big)
            nc.vector.tensor_mul(g[:], z[:], d[:])
            o = tmp("o", pool=pool_out)
            nc.vector.tensor_add(o[:], t4[:], g[:])

            dst = out_ap[b, :, q, :].rearrange("p (w c) -> p w c", c=ch)
            nc.sync.dma_start(
                dst[:, w_lo : w_lo + nw, :],
                o[:].rearrange("p (w c) -> p w c", c=ch),
            )

        wo_q = wq // 2  # output w-pairs per quarter
        n_macro = bpc * nq
        tiles = []
        for b in range(bpc):
            for qq in range(nq):
                first = not tiles
                last = b == bpc - 1 and qq == nq - 1
                if first:
                    tiles.append((b, qq, 0, wo_q // 4))
                    tiles.append((b, qq, wo_q // 4, wo_q // 2))
                    tiles.append((b, qq, wo_q // 2, wo_q))
                elif last:
                    tiles.append((b, qq, 0, wo_q // 2))
                    tiles.append((b, qq, wo_q // 2, 3 * wo_q // 4))
                    tiles.append((b, qq, 3 * wo_q // 4, wo_q))
                else:
                    tiles.append((b, qq, 0, wo_q))
        pending = emit_load(*tiles[0], dve_cast=True)
        for i in range(len(tiles)):
            nxt = (
                emit_load(*tiles[i + 1], dve_cast=(i <= 1))
                if i + 1 < len(tiles)
                else None
            )
            emit_compute(pending, ramp=(i < 3))
            pending = nxt

    nc.compile()
    return nc


def _get_program(bpc, ho, nq, wq, ch, plan):
    key = (bpc, ho, nq, wq, ch, plan)
    if key not in _PROGRAM_CACHE:
        _PROGRAM_CACHE[key] = _build_program(bpc, ho, nq, wq, ch, plan)
    return _PROGRAM_CACHE[key]


def _mask_plan(mask):
    """Derive the plan so every ratio has |r| <= 1.

    xs = m00*Ee + m01*Eo + m10*Oe + m11*Oo
       = cE*(rE*Ea + Eb) + cO*(rO*Oa + Ob)
    with (Ea, Eb) = (Ee, Eo) or swapped so |rE| <= 1 (cE = the larger-|.|
    coefficient of the E pair), likewise the O pair. Final:
    xsp = rB*Fa + Fb with Fb the pair whose carrier |c| is larger;
    f = carrier of Fb, rB = other carrier / f.
    """
    m = np.asarray(mask, np.float64).reshape(-1)  # m00, m01, m10, m11
    mE = (m[0], m[1])
    mO = (m[2], m[3])

    def pair(coeffs):
        ca, cb = coeffs  # coeff of the 'even' slice, coeff of the 'odd' slice
        # swapped=False: A' = r*even + odd, carrier = cb (odd coeff), r = ca/cb
        # swapped=True:  A' = r*odd + even, carrier = ca, r = cb/ca
        if abs(ca) <= abs(cb):
            carrier = cb
            r = ca / cb if cb != 0.0 else 0.0
            return False, r, carrier
        carrier = ca
        r = cb / ca if ca != 0.0 else 0.0
        return True, r, carrier

    swapE, rE, cE = pair(mE)
    swapO, rO, cO = pair(mO)
    # xs = cE*A' + cO*B'; scale the smaller carrier
    if abs(cO) <= abs(cE):
        swapF = False  # scale B' (Fa = Bp), carrier f = cE
        f = cE
        rB = cO / cE if cE != 0.0 else 0.0
    else:
        swapF = True   # scale A'
        f = cO
        rB = cE / cO if cO != 0.0 else 0.0

    # ratios ride as float32 instruction immediates; round-trip through
    # float32 so the compile-key is exactly what the program computes
    rE32, rO32, rB32, f32v = (np.float32(v) for v in (rE, rO, rB, f))
    return (
        swapE, swapO, swapF,
        float(rE32), float(rO32), float(rB32), float(f32v),
    )


def kernel(x, mask):
    import os

    global LAST_EXEC_NS, LAST_RESULTS

    x = np.asarray(x)
    mask = np.asarray(mask)
    assert x.shape == (B, H, W, C), x.shape
    in_dtype = x.dtype

    plan = _mask_plan(mask)
    nc = _get_program(BPC, HO, NQ, WQ, C, plan)

    xv = np.ascontiguousarray(x, np.float32).reshape(B, HO, 2, NQ, WQ * C)

    in_maps = [
        {"x": xv[i * BPC : (i + 1) * BPC]} for i in range(N_CORES)
    ]

    trace = os.environ.get("KERNEL_TRACE", "0") == "1"
    res = run_bass_kernel_spmd(
        nc, in_maps, core_ids=list(range(N_CORES)), trace=trace
    )
    LAST_EXEC_NS = res.exec_time_ns
    LAST_RESULTS = res

    parts = [
        r["out"].reshape(BPC, HO, NQ, WQ // 2, C).reshape(BPC, HO, W // 2, C)
        for r in res.results
    ]
    full = np.concatenate(parts, axis=0)
    return full.astype(in_dtype, copy=False)


def _numpy_reference(x, mask):
    xr = x.reshape(x.shape[0], x.shape[1] // 2, 2, x.shape[2] // 2, 2, x.shape[3])
    x1 = xr.max(axis=(2, 4))
    x2 = xr.mean(axis=(2, 4))
    xs = np.einsum("bhiwjc,ij->bhwc", xr, mask)
    z = 1.0 / (1.0 + np.exp(-xs))
    return z * x1 + (1.0 - z) * x2


if __name__ == "__main__":
    # Small-scale CoreSim self-test (no hardware needed).
    from concourse.bass_interp import CoreSim

    rng = np.random.default_rng(0)
    for trial in range(4):
        bpc_s, nq_s, wq_s = 1, 1, 8
        h_s, w_s = 256, nq_s * wq_s
        xs_np = rng.standard_normal((bpc_s, h_s, w_s, C)).astype(np.float32)
        mask_np = (rng.standard_normal((2, 2)) * 0.5).astype(np.float32)

        plan_s = _mask_plan(mask_np)
        nc = _build_program(bpc_s, 128, nq_s, wq_s, C, plan_s)
        sim = CoreSim(nc, trace=False)
        sim.tensor("x")[:] = xs_np.reshape(bpc_s, 128, 2, nq_s, wq_s * C)
        sim.simulate()
        got = (
            sim.tensor("out")
            .astype(np.float64)
            .reshape(bpc_s, 128, nq_s, wq_s // 2, C)
            .reshape(bpc_s, 128, w_s // 2, C)
        )
        want = _numpy_reference(xs_np.astype(np.float64), mask_np.astype(np.float64))
        err = np.abs(got - want)
        rel = err.max() / np.abs(want).max()
        print(f"trial {trial} plan={plan_s} mask={mask_np.reshape(-1)} "
              f"max abs {err.max():.2e} rel {rel:.2e}")
        assert rel < 5e-3, rel
    print("PASS")



# revision 32
# speedup vs baseline: 1.1860x; 1.0127x over previous
"""Gated max/avg 2x2 pooling kernel for Trainium2 (8 NeuronCores, SPMD) — v3.7.

Reference computation (per 2x2 window over [B, H, W, C], stride 2):
    x1 = max(window), x2 = mean(window)
    xs = sum_ij mask[i, j] * window[i, j]   (per channel)
    z  = sigmoid(xs)
    out = z * x1 + (1 - z) * x2

Engine split (all scalar operands ride as instruction immediates):
  ACT : f32->f16 deinterleaving cast (2 instr/tile), q3 = rB*Fa,
        t4 = 0.25*s, sigmoid
  DVE : max tree (2 TT), sum tree (2 TT), xs pair-Horner
        (2 TS + fused (Ap||Bp) TT + xsp TT), d/g/o blend (3 TT)
  DMA : f32 in via separate per-parity staging tiles (each frees for
        the i+2 prefetch at its own cast) on the sync HWDGE ring; f16
        out issued via nc.scalar.dma_start so stores ride ACT's
        separate HWDGE ring instead of queueing behind 4MB of input
        per tile on the single sync logical queue (host upcasts)

xs pair-Horner: A' = rE*Ea + Eb, B' = rO*Oa + Ob computed as ONE
TT@2*fdo against a strided (Eb||Ob) view; xsp = rB*Fa + Fb;
z = sigmoid(f * xsp). The (a, b) slice order is chosen on the host so
every ratio has |r| <= 1; f is the remaining carrier scale, applied
for free by the ACT sigmoid's scale immediate.

Perf notes (hardware-measured; see memory/trn2-measured-facts): the
kernel is DVE-throughput-bound (~129us busy), not DMA-bound (~105us).
GPSIMD elementwise ops slow concurrent DVE ops ~2.7x via the shared
SBUF port, so GPSIMD stays idle on purpose. DVE TT on 64-element-run
strided views runs at full packed rate. Rejected by interleaved A/B:
w-chunked DMA+cast pipeline, q1/q2 scales on ACT, software-pipelined
blend (one tile behind), folding 0.25 into the cast scale (wrong
math). The DVE clock varies ~20% between runs (p-state); compare
timings via the median TT@2048 duration (~1218ns fast), and use
measure_ab.py (interleaved A/B) since machine drift between batches
exceeds most effect sizes.
"""

import numpy as np

import concourse.bacc as bacc
import concourse.mybir as mybir
import concourse.tile as tile
from concourse.bass_utils import run_bass_kernel_spmd

F32 = mybir.dt.float32
F16 = mybir.dt.float16

B, H, W, C = 16, 256, 256, 64
N_CORES = 8
BPC = B // N_CORES          # batches per core
HO = H // 2                 # 128 output rows = SBUF partitions
NQ = 4                      # w-quarters per row
WQ = W // NQ                # input w per macro-tile (64)

LAST_EXEC_NS = None
LAST_RESULTS = None

_PROGRAM_CACHE = {}


def _build_program(bpc, ho, nq, wq, ch, plan):
    """Build + compile the single-core Bass/Tile program (SPMD-shared).

    plan = (swapE, swapO, swapF, rE, rO, rB, f): slice assignment for the
    pair-Horner plus the ratio/carrier values, all baked as instruction
    immediates (skips the scalar-value loads a PTR operand needs).
    """
    from contextlib import ExitStack

    assert ho == 128, "partition dim must be 128"
    swapE, swapO, swapF, rE, rO, rB, fca = plan
    fd_in = wq * ch            # free dim of an E/O tile (4096)
    wo = wq // 2               # output w per macro-tile
    fd_out = wo * ch           # free dim of output tile (2048)

    nc = bacc.Bacc(
        "TRN2",
        target_bir_lowering=False,
        debug=False,
        enable_asserts=False,
        num_devices=N_CORES,
    )

    x = nc.dram_tensor("x", [bpc, ho, 2, nq, fd_in], F32, kind="ExternalInput")
    out = nc.dram_tensor("out", [bpc, ho, nq, fd_out], F16, kind="ExternalOutput")
    x_ap = x.ap()
    out_ap = out.ap()

    alu = mybir.AluOpType

    with tile.TileContext(nc) as tc, ExitStack() as ctx:
        pool_io = ctx.enter_context(tc.tile_pool(name="io", bufs=2))
        pool_y = ctx.enter_context(tc.tile_pool(name="y16", bufs=2))
        pool_big = ctx.enter_context(tc.tile_pool(name="big", bufs=1))
        pool_tmp = ctx.enter_context(tc.tile_pool(name="tmp", bufs=1))
        pool_gps = ctx.enter_context(tc.tile_pool(name="gps", bufs=2))
        pool_out = ctx.enter_context(tc.tile_pool(name="outp", bufs=2))

        def emit_load(b, q, w_lo, w_hi, dve_cast=False):
            """Stage 1: input DMA + ACT cast for one tile.

            dma_ways: extra w-wise split of each row-parity's DMA; >1 for
            the ramp tiles, where single-queue DMA latency gates startup."""
            nw = w_hi - w_lo
            fde = nw * 2 * ch
            # separate per-parity staging tiles: each frees for the i+2
            # prefetch as soon as ITS cast is done, not both
            E32 = pool_io.tile([128, fde], F32, tag="E32")
            O32 = pool_io.tile([128, fde], F32, tag="O32")
            src = x_ap[b, :, :, q, :].rearrange(
                "p r (w c) -> p r w c", c=2 * ch
            )[:, :, w_lo : w_lo + nw, :]
            nc.sync.dma_start(
                E32[:].rearrange("p (w c) -> p w c", c=2 * ch), src[:, 0])
            nc.sync.dma_start(
                O32[:].rearrange("p (w c) -> p w c", c=2 * ch), src[:, 1])
            Y = pool_y.tile([128, 2 * fde], F16, tag="Y")
            # cast + column-deinterleave: write order (r, e, w, c) while
            # reading DMA order (r, w, e, c); downstream slices contiguous.
            # One cast per row-parity keeps the APs within 3 free dims.
            Yw = Y[:].rearrange(
                "p (r e w c) -> p r w e c", r=2, e=2, c=ch
            )
            Ein = E32[:].rearrange("p (w e c) -> p w e c", e=2, c=ch)
            Oin = O32[:].rearrange("p (w e c) -> p w e c", e=2, c=ch)
            if dve_cast:
                # ramp tiles: DVE is idle waiting on this cast anyway, and
                # casts ~1.7x faster than ACT; skips two cross-engine hops
                nc.vector.tensor_copy(Yw[:, 0], Ein)
                nc.vector.tensor_copy(Yw[:, 1], Oin)
            else:
                nc.scalar.copy(Yw[:, 0], Ein)
                nc.scalar.copy(Yw[:, 1], Oin)
            return dict(b=b, q=q, w_lo=w_lo, nw=nw, fde=fde, fdo=nw * ch, Y=Y)

        def emit_compute(h, ramp=False):
            """Stage 2: DVE trees + sigmoid + GPS blend + output DMA."""
            b, q, w_lo, nw = h["b"], h["q"], h["w_lo"], h["nw"]
            fde, fdo, Y = h["fde"], h["fdo"], h["Y"]

            def tmp(tag, fd=fdo, dt=F16, pool=pool_tmp):
                t = pool.tile([128, fd], dt, tag=tag)
                return t

            Ef = Y[:, 0:fde]
            Of = Y[:, fde : 2 * fde]
            # deinterleaved layout: all four window slices are contiguous
            Ee, Eo = Y[:, 0:fdo], Y[:, fdo : 2 * fdo]
            Oe, Oo = Y[:, fde : fde + fdo], Y[:, fde + fdo : 2 * fde]

            def v(t):
                return t[:].rearrange("p (w c) -> p w c", c=ch)

            # xs chain first so the sigmoid's input is ready early; the
            # independent max/sum trees fill the ACT-latency windows
            Ea, Eb = (Eo, Ee) if swapE else (Ee, Eo)
            Oa, Ob = (Oo, Oe) if swapO else (Oe, Oo)
            # q12 = (rE*Ea || rO*Oa); one TT then adds (Eb || Ob), so the
            # two pair-combines share a single @2*fdo instruction
            q12 = tmp("q12", fd=fde, pool=pool_gps)
            nc.vector.tensor_scalar_mul(q12[:, 0:fdo], Ea, float(rE))
            nc.vector.tensor_scalar_mul(q12[:, fdo:fde], Oa, float(rO))
            # (Eb || Ob) as one strided view over Y's four fdo-slices
            sEb = (0 if swapE else 1)
            sOb = 2 + (0 if swapO else 1)
            Ys = Y[:].rearrange("p (s x) -> p s x", s=4)
            ebob = Ys[:, slice(sEb, sOb + 1, sOb - sEb)]
            AB = tmp("AB", fd=fde, pool=pool_gps)
            nc.vector.tensor_add(
                AB[:].rearrange("p (s x) -> p s x", s=2), q12[:].rearrange(
                    "p (s x) -> p s x", s=2), ebob,
            )
            Ap = AB[:, 0:fdo]
            Bp = AB[:, fdo:fde]
            Fa, Fb = (Ap, Bp) if swapF else (Bp, Ap)
            q3 = tmp("q3", pool=pool_gps)
            if ramp:
                nc.vector.tensor_scalar_mul(q3[:], Fa, float(rB))
            else:
                nc.scalar.mul(q3[:], Fa, float(rB))

            # big vertical combines run while ACT computes q3
            M1 = tmp("M1", fd=fde, pool=pool_big)
            nc.vector.tensor_max(M1[:], Ef, Of)
            S1 = tmp("S1", fd=fde, pool=pool_big)
            nc.vector.tensor_add(S1[:], Ef, Of)

            xsp = tmp("xsp", pool=pool_gps)
            nc.vector.tensor_add(xsp[:], q3[:], Fb)
            z = tmp("z", pool=pool_gps)
            nc.scalar.activation(
                z[:], xsp[:],
                mybir.ActivationFunctionType.Sigmoid,
                scale=float(fca),
            )

            # horizontal reduces + blend while ACT computes the sigmoid
            x1 = tmp("x1")
            nc.vector.tensor_max(x1[:], M1[:, 0:fdo], M1[:, fdo:fde])
            s = tmp("s", pool=pool_big)
            nc.vector.tensor_add(s[:], S1[:, 0:fdo], S1[:, fdo:fde])
            t4 = tmp("t4", pool=pool_gps)
            if ramp:
                nc.vector.tensor_scalar_mul(t4[:], s[:], 0.25)
            else:
                nc.scalar.mul(t4[:], s[:], 0.25)
            d = tmp("d", pool=pool_gps)
            nc.vector.tensor_sub(d[:], x1[:], t4[:])
            g = tmp("g", pool=pool_big)
            nc.vector.tensor_mul(g[:], z[:], d[:])
            o = tmp("o", pool=pool_out)
            nc.vector.tensor_add(o[:], t4[:], g[:])

            dst = out_ap[b, :, q, :].rearrange("p (w c) -> p w c", c=ch)
            # issue the store on ACT's HWDGE ring (qActDynamicHW): all
            # sync-issued DMAs share one logical queue, so a store issued
            # there queues behind the next tiles' 4MB of input transfers
            nc.scalar.dma_start(
                dst[:, w_lo : w_lo + nw, :],
                o[:].rearrange("p (w c) -> p w c", c=ch),
            )

        wo_q = wq // 2  # output w-pairs per quarter
        n_macro = bpc * nq
        tiles = []
        for b in range(bpc):
            for qq in range(nq):
                first = not tiles
                last = b == bpc - 1 and qq == nq - 1
                if first:
                    tiles.append((b, qq, 0, wo_q // 4))
                    tiles.append((b, qq, wo_q // 4, wo_q // 2))
                    tiles.append((b, qq, wo_q // 2, wo_q))
                elif last:
                    tiles.append((b, qq, 0, wo_q // 2))
                    tiles.append((b, qq, wo_q // 2, 3 * wo_q // 4))
                    tiles.append((b, qq, 3 * wo_q // 4, wo_q))
                else:
                    tiles.append((b, qq, 0, wo_q))
        pending = emit_load(*tiles[0], dve_cast=True)
        for i in range(len(tiles)):
            nxt = (
                emit_load(*tiles[i + 1], dve_cast=(i <= 1))
                if i + 1 < len(tiles)
                else None
            )
            emit_compute(pending, ramp=(i < 3))
            pending = nxt

    nc.compile()
    return nc


def _get_program(bpc, ho, nq, wq, ch, plan):
    key = (bpc, ho, nq, wq, ch, plan)
    if key not in _PROGRAM_CACHE:
        _PROGRAM_CACHE[key] = _build_program(bpc, ho, nq, wq, ch, plan)
    return _PROGRAM_CACHE[key]


def _mask_plan(mask):
    """Derive the plan so every ratio has |r| <= 1.

    xs = m00*Ee + m01*Eo + m10*Oe + m11*Oo
       = cE*(rE*Ea + Eb) + cO*(rO*Oa + Ob)
    with (Ea, Eb) = (Ee, Eo) or swapped so |rE| <= 1 (cE = the larger-|.|
    coefficient of the E pair), likewise the O pair. Final:
    xsp = rB*Fa + Fb with Fb the pair whose carrier |c| is larger;
    f = carrier of Fb, rB = other carrier / f.
    """
    m = np.asarray(mask, np.float64).reshape(-1)  # m00, m01, m10, m11
    mE = (m[0], m[1])
    mO = (m[2], m[3])

    def pair(coeffs):
        ca, cb = coeffs  # coeff of the 'even' slice, coeff of the 'odd' slice
        # swapped=False: A' = r*even + odd, carrier = cb (odd coeff), r = ca/cb
        # swapped=True:  A' = r*odd + even, carrier = ca, r = cb/ca
        if abs(ca) <= abs(cb):
            carrier = cb
            r = ca / cb if cb != 0.0 else 0.0
            return False, r, carrier
        carrier = ca
        r = cb / ca if ca != 0.0 else 0.0
        return True, r, carrier

    swapE, rE, cE = pair(mE)
    swapO, rO, cO = pair(mO)
    # xs = cE*A' + cO*B'; scale the smaller carrier
    if abs(cO) <= abs(cE):
        swapF = False  # scale B' (Fa = Bp), carrier f = cE
        f = cE
        rB = cO / cE if cE != 0.0 else 0.0
    else:
        swapF = True   # scale A'
        f = cO
        rB = cE / cO if cO != 0.0 else 0.0

    # ratios ride as float32 instruction immediates; round-trip through
    # float32 so the compile-key is exactly what the program computes
    rE32, rO32, rB32, f32v = (np.float32(v) for v in (rE, rO, rB, f))
    return (
        swapE, swapO, swapF,
        float(rE32), float(rO32), float(rB32), float(f32v),
    )


def kernel(x, mask):
    import os

    global LAST_EXEC_NS, LAST_RESULTS

    x = np.asarray(x)
    mask = np.asarray(mask)
    assert x.shape == (B, H, W, C), x.shape
    in_dtype = x.dtype

    plan = _mask_plan(mask)
    nc = _get_program(BPC, HO, NQ, WQ, C, plan)

    xv = np.ascontiguousarray(x, np.float32).reshape(B, HO, 2, NQ, WQ * C)

    in_maps = [
        {"x": xv[i * BPC : (i + 1) * BPC]} for i in range(N_CORES)
    ]

    trace = os.environ.get("KERNEL_TRACE", "0") == "1"
    res = run_bass_kernel_spmd(
        nc, in_maps, core_ids=list(range(N_CORES)), trace=trace
    )
    LAST_EXEC_NS = res.exec_time_ns
    LAST_RESULTS = res

    parts = [
        r["out"].reshape(BPC, HO, NQ, WQ // 2, C).reshape(BPC, HO, W // 2, C)
        for r in res.results
    ]
    full = np.concatenate(parts, axis=0)
    return full.astype(in_dtype, copy=False)


def _numpy_reference(x, mask):
    xr = x.reshape(x.shape[0], x.shape[1] // 2, 2, x.shape[2] // 2, 2, x.shape[3])
    x1 = xr.max(axis=(2, 4))
    x2 = xr.mean(axis=(2, 4))
    xs = np.einsum("bhiwjc,ij->bhwc", xr, mask)
    z = 1.0 / (1.0 + np.exp(-xs))
    return z * x1 + (1.0 - z) * x2


if __name__ == "__main__":
    # Small-scale CoreSim self-test (no hardware needed).
    from concourse.bass_interp import CoreSim

    rng = np.random.default_rng(0)
    for trial in range(4):
        bpc_s, nq_s, wq_s = 1, 1, 8
        h_s, w_s = 256, nq_s * wq_s
        xs_np = rng.standard_normal((bpc_s, h_s, w_s, C)).astype(np.float32)
        mask_np = (rng.standard_normal((2, 2)) * 0.5).astype(np.float32)

        plan_s = _mask_plan(mask_np)
        nc = _build_program(bpc_s, 128, nq_s, wq_s, C, plan_s)
        sim = CoreSim(nc, trace=False)
        sim.tensor("x")[:] = xs_np.reshape(bpc_s, 128, 2, nq_s, wq_s * C)
        sim.simulate()
        got = (
            sim.tensor("out")
            .astype(np.float64)
            .reshape(bpc_s, 128, nq_s, wq_s // 2, C)
            .reshape(bpc_s, 128, w_s // 2, C)
        )
        want = _numpy_reference(xs_np.astype(np.float64), mask_np.astype(np.float64))
        err = np.abs(got - want)
        rel = err.max() / np.abs(want).max()
        print(f"trial {trial} plan={plan_s} mask={mask_np.reshape(-1)} "
              f"max abs {err.max():.2e} rel {rel:.2e}")
        assert rel < 5e-3, rel
    print("PASS")

